# revision 22
# baseline (speedup 1.0000x reference)
"""CrossFuse kernel for Trainium2 (Bass/Tile), data-parallel over batch.

Math per sample (c=2048 channels, n=1024 spatial):
  e1,e2: (c,n);  s_i = softmax(e_i, axis=-1);  m_i = mean(e_i, axis=-1)
  inner1 = e1/n + m2*s1 ;  inner2 = s2*m1 + e2/n
  embI1 = e1*(1+inner1) ; embI2 = e2*(1+inner2)
  y = mean(concat(embI1, embI2), spatial)              # (4096,)
  hid = relu(w1 @ y); mask = sigmoid(w2 @ hid)         # (256,), (4096,)
  out = concat(embI1, embI2) * (1 + mask[c])

Per-core kernel (1 sample/core): both inputs SBUF-resident; per channel-tile
the ScalarE computes exp(E) and E/n+1 with free row-sum accumulation (softmax
denominator + mean), VectorE fuses W=X*inv+P (scalar_tensor_tensor) and
E_out=E*W with row-sum -> y (affine_mul_reduce, custom DVE op — the ISA
tensor_tensor_reduce crashes TRN2 here). SE FCs run on TensorE with
host-pre-transposed weights; sigmoid via tanh (shares exp's ACT table set).
"""

from contextlib import ExitStack

import numpy as np

import concourse.bacc as bacc
import concourse.tile as tile
from concourse import mybir
from concourse.bass_utils import run_bass_kernel_spmd

B, C, H, W_SP = 8, 2048, 32, 32
N = H * W_SP  # 1024
CT = C // 128  # 16 channel tiles per input tensor
NT = 2 * CT  # 32 total channel tiles / mask chunks
CH2 = 2 * C  # 4096
RED = 256
NCORES = 8

F32 = mybir.dt.float32
AF = mybir.ActivationFunctionType
ALU = mybir.AluOpType


def _body(tc, e1_d, e2_d, w1t_d, w2t_d, out_d, use_fc=True):
    nc = tc.nc
    with ExitStack() as ctx:
        ep = ctx.enter_context(tc.tile_pool(name="emb", bufs=1))
        wp = ctx.enter_context(tc.tile_pool(name="weights", bufs=1))
        w1p = ctx.enter_context(tc.tile_pool(name="w1chunk", bufs=3))
        sp = ctx.enter_context(tc.tile_pool(name="scratch", bufs=2))
        st = ctx.enter_context(tc.tile_pool(name="stats", bufs=1))
        ob = ctx.enter_context(tc.tile_pool(name="outbuf", bufs=3))
        pp = ctx.enter_context(tc.tile_pool(name="psum", bufs=1, space="PSUM"))

        E1 = ep.tile([128, CT * N], F32, name="E1")
        E2 = ep.tile([128, CT * N], F32, name="E2")
        w2t_sb = wp.tile([128, 2 * CH2], F32, name="w2t_sb")

        ys = st.tile([128, NT], F32, name="ys")
        Zs = st.tile([128, NT], F32, name="Zs")
        As = st.tile([128, NT], F32, name="As")
        Rz = st.tile([128, NT], F32, name="Rz")
        Inv = st.tile([128, NT], F32, name="Inv")
        hid_sb = st.tile([128, 2], F32, name="hid_sb")
        scale_sb = st.tile([128, NT], F32, name="scale_sb")

        hidA = pp.tile([128, 1], F32, name="hidA")
        hidB = pp.tile([128, 1], F32, name="hidB")
        maskp = pp.tile([128, NT], F32, name="maskp")

        # Stream inputs per channel-tile so compute starts on the first tile.
        for t in range(CT):
            nc.sync.dma_start(E1[:, t * N : (t + 1) * N], e1_d[t * 128 : (t + 1) * 128, :])
            nc.sync.dma_start(E2[:, t * N : (t + 1) * N], e2_d[t * 128 : (t + 1) * 128, :])

        mm = 0
        for t in range(CT):
            s1 = E1[:, t * N : (t + 1) * N]
            s2 = E2[:, t * N : (t + 1) * N]
            c1, c2 = t, CT + t  # global chunk columns for e1/e2 stats

            X1 = sp.tile([128, N], F32, name="X1", tag="X1")
            P1 = sp.tile([128, N], F32, name="P1", tag="P1")
            X2 = sp.tile([128, N], F32, name="X2", tag="X2")
            P2 = sp.tile([128, N], F32, name="P2", tag="P2")

            # X = exp(E), Z = rowsum(X);  P = E/n + 1, A = rowsum(P) = mean + n
            nc.scalar.activation(X1[:], s1, AF.Exp, accum_out=Zs[:, c1 : c1 + 1])
            nc.scalar.activation(
                P1[:], s1, AF.Identity, bias=1.0, scale=1.0 / N,
                accum_out=As[:, c1 : c1 + 1],
            )
            nc.scalar.activation(X2[:], s2, AF.Exp, accum_out=Zs[:, c2 : c2 + 1])
            nc.scalar.activation(
                P2[:], s2, AF.Identity, bias=1.0, scale=1.0 / N,
                accum_out=As[:, c2 : c2 + 1],
            )

            nc.vector.reciprocal(Rz[:, c1 : c1 + 1], Zs[:, c1 : c1 + 1])
            nc.vector.reciprocal(Rz[:, c2 : c2 + 1], Zs[:, c2 : c2 + 1])
            # inv1 = mean(e2)/Z1 = (A2 - n) * (1/Z1); inv2 = (A1 - n) * (1/Z2)
            nc.vector.scalar_tensor_tensor(
                Inv[:, c1 : c1 + 1], As[:, c2 : c2 + 1], float(N),
                Rz[:, c1 : c1 + 1], op0=ALU.subtract, op1=ALU.mult,
            )
            nc.vector.scalar_tensor_tensor(
                Inv[:, c2 : c2 + 1], As[:, c1 : c1 + 1], float(N),
                Rz[:, c2 : c2 + 1], op0=ALU.subtract, op1=ALU.mult,
            )

            # W = X*inv + P (in-place over X); then E = W*E with rowsum -> ys
            # (affine_mul_reduce: out = (in0*scale+bias)*in1, accum = rowsum)
            nc.vector.scalar_tensor_tensor(
                X1[:], X1[:], Inv[:, c1 : c1 + 1], P1[:], op0=ALU.mult, op1=ALU.add
            )
            nc.vector.affine_mul_reduce(
                out=s1, accum_out=ys[:, c1 : c1 + 1], in0=X1[:], in1=s1,
                scale=1.0, bias=0.0,
            )
            nc.vector.scalar_tensor_tensor(
                X2[:], X2[:], Inv[:, c2 : c2 + 1], P2[:], op0=ALU.mult, op1=ALU.add
            )
            nc.vector.affine_mul_reduce(
                out=s2, accum_out=ys[:, c2 : c2 + 1], in0=X2[:], in1=s2,
                scale=1.0, bias=0.0,
            )

            # FC1 accumulation: hid += w1t[chunk].T @ ys[chunk]
            for c in (c1, c2) if use_fc else ():
                w1c = w1p.tile([128, RED], F32, name="w1c", tag="w1c")
                nc.sync.dma_start(w1c[:], w1t_d[c * 128 : (c + 1) * 128, :])
                nc.tensor.matmul(
                    hidA[:], w1c[:, 0:128], ys[:, c : c + 1],
                    start=(mm == 0), stop=(mm == NT - 1),
                )
                nc.tensor.matmul(
                    hidB[:], w1c[:, 128:256], ys[:, c : c + 1],
                    start=(mm == 0), stop=(mm == NT - 1),
                )
                mm += 1

        if use_fc:
            # w2t resident (emitted late; only FC2 depends on it)
            nc.sync.dma_start(w2t_sb[:, 0:CH2], w2t_d[0:128, :])
            nc.sync.dma_start(w2t_sb[:, CH2 : 2 * CH2], w2t_d[128:256, :])

            nc.scalar.activation(hid_sb[:, 0:1], hidA[:], AF.Relu)
            nc.scalar.activation(hid_sb[:, 1:2], hidB[:], AF.Relu)

            # FC2: mask_pre[chunk] = w2[chunk,:] @ hid   (lhsT = w2t slices)
            for c in range(NT):
                nc.tensor.matmul(
                    maskp[:, c : c + 1], w2t_sb[:, c * 128 : (c + 1) * 128],
                    hid_sb[:, 0:1], start=True, stop=False,
                )
                nc.tensor.matmul(
                    maskp[:, c : c + 1], w2t_sb[:, CH2 + c * 128 : CH2 + (c + 1) * 128],
                    hid_sb[:, 1:2], start=False, stop=True,
                )

            # 1 + sigmoid(x) = 1.5 + 0.5*tanh(x/2)  (tanh shares exp's table set)
            nc.scalar.activation(scale_sb[:], maskp[:], AF.Tanh, scale=0.5)
            nc.vector.tensor_scalar(
                scale_sb[:], scale_sb[:], 0.5, 1.5, op0=ALU.mult, op1=ALU.add
            )
        else:
            nc.vector.memset(scale_sb[:], 1.7)

        for t in range(CT):
            for Ebuf, col in ((E1, t), (E2, CT + t)):
                o = ob.tile([128, N], F32, name="obuf", tag="obuf")
                nc.vector.tensor_scalar(
                    o[:], Ebuf[:, t * N : (t + 1) * N],
                    scale_sb[:, col : col + 1], None, op0=ALU.mult,
                )
                nc.sync.dma_start(out_d[col * 128 : (col + 1) * 128, :], o[:])


_NC_CACHE = {}


def _get_nc(use_fc=True):
    key = ("nc", use_fc)
    if key not in _NC_CACHE:
        nc = bacc.Bacc(
            "TRN2",
            target_bir_lowering=False,
            debug=False,
            enable_asserts=False,
            num_devices=NCORES,
        )
        e1_d = nc.dram_tensor("emb1", (C, N), F32, kind="ExternalInput").ap()
        e2_d = nc.dram_tensor("emb2", (C, N), F32, kind="ExternalInput").ap()
        w1t_d = nc.dram_tensor("w1t", (CH2, RED), F32, kind="ExternalInput").ap()
        w2t_d = nc.dram_tensor("w2t", (RED, CH2), F32, kind="ExternalInput").ap()
        out_d = nc.dram_tensor("out", (CH2, N), F32, kind="ExternalOutput").ap()
        with tile.TileContext(nc) as tc:
            _body(tc, e1_d, e2_d, w1t_d, w2t_d, out_d, use_fc=use_fc)
        nc.compile()
        _NC_CACHE[key] = nc
    return _NC_CACHE[key]


def make_in_maps(emb1, emb2, w1, w2):
    w1t = np.ascontiguousarray(w1.T) / np.float32(N)
    w2t = np.ascontiguousarray(w2.T)
    return [
        {
            "emb1": np.ascontiguousarray(emb1[i].reshape(C, N)),
            "emb2": np.ascontiguousarray(emb2[i].reshape(C, N)),
            "w1t": w1t,
            "w2t": w2t,
        }
        for i in range(B)
    ]


def run(emb1, emb2, w1, w2, trace=False):
    """Returns (output, BassKernelResults)."""
    nc = _get_nc()
    in_maps = make_in_maps(emb1, emb2, w1, w2)
    res = run_bass_kernel_spmd(nc, in_maps, list(range(NCORES)), trace=trace)
    out = np.stack(
        [res.results[i]["out"].reshape(CH2, H, W_SP) for i in range(B)]
    )
    return out, res


def kernel(emb1, emb2, w1, w2):
    out, _ = run(
        np.asarray(emb1), np.asarray(emb2), np.asarray(w1), np.asarray(w2)
    )
    return out
